# revision 35
# baseline (speedup 1.0000x reference)
"""MoE-routing actor kernel for 8 Trainium2 NeuronCores.

Strategy (pure data parallel, expert-sorted, bf16 compute):
  - Host: for each expert m, deal its rows round-robin to the 8 cores so all
    cores get near-identical per-expert counts and can share ONE SPMD graph.
    Per-expert row capacities are the max count over cores (row-granular,
    <1% padding); rows are packed sorted-by-expert.
  - The tiny shared trunk (fc1: 262144x32 @ 32x34 + relu, ~0.6 GFLOP) runs on
    host BLAS; the device gets pre-packed transposed activations with an
    all-ones row 34 that folds the expert bias bout into the expert matmul.
  - Mask applied host-side: the device computes only the first A_DEV (<=128)
    kept output columns; masked columns are exact -1e9 filled host-side and
    kept columns beyond A_DEV (typically ~9 of 137) are computed on host.
  - Device (raw bacc, manual semaphores, no Tile framework): per 1024-row
    super-chunk, transposed expert matmuls (stationary weff_e [35, A_DEV],
    moving activation run of <=512 rows, expert-boundary runs split).
    Consecutive 512-row halves alternate PE partition base 0/64 so each
    LDWEIGHTS targets row-strips disjoint from the in-flight matmul AND the
    two halves' matmuls execute concurrently on disjoint sub-arrays
    (~0.5 PE cycles/row at the fixed 1.2 GHz clock).  PSUM->bf16 casts
    alternate VectorE/ScalarE per super; stores go out in 512KB pairs on the
    sync HWDGE ring; input loads stream on the gpsimd queue (group 0 on the
    scalar ring to dodge SWDGE boot latency).
"""

import os
import sys

sys.path.insert(0, "/opt/trn_rl_repo")

import numpy as np
import ml_dtypes

BF16 = ml_dtypes.bfloat16

B = 262144
NCORES = 8
J = 16
M = 12
H = 34
HP = H + 1  # fc1 output + ones row for bias folding
S_DIM = 32  # state dim
A = J * J  # 256 action logits
NEG = np.float32(-1.0e9)
SUPER = 1024  # rows per compute chunk
HALF = 512  # PSUM-bank / matmul free-dim granule
NP = 4  # psum ring depth (supers)

_BUILD_CACHE: dict = {}
LAST_RESULT = None  # BassKernelResults of the most recent run (for profiling)


def _make_runs(caps, R):
    """Per 512-row half-chunk, the (expert, row0, row1) runs covering it."""
    offs = np.concatenate([[0], np.cumsum(caps)])
    assert offs[-1] == R
    runs = [[] for _ in range(R // HALF)]
    for m in range(len(caps)):
        lo, hi = int(offs[m]), int(offs[m + 1])
        if lo >= hi:
            continue
        for g in range(lo // HALF, (hi - 1) // HALF + 1):
            a = max(lo, g * HALF)
            b = min(hi, (g + 1) * HALF)
            if a < b:
                runs[g].append((m, a, b))
    return runs


def _build(R: int, caps: tuple, Adev: int):
    """Raw-bacc device graph: manual semaphores, static SBUF allocation."""
    from concourse import bacc, mybir

    runs = _make_runs(list(caps), R)
    f32 = mybir.dt.float32
    bf16 = mybir.dt.bfloat16
    nc = bacc.Bacc("TRN2", target_bir_lowering=False, debug=False)

    n_super = R // SUPER
    GRP = 3 if n_super % 3 == 0 else (2 if n_super % 2 == 0 else 1)
    n_grp = n_super // GRP
    GCOL = GRP * HALF
    n_pair = (n_super + 1) // 2

    xat_d = nc.declare_dram_parameter("xat", [n_grp, 2, HP, GCOL], bf16, isOutput=False)
    weff_d = nc.declare_dram_parameter("weff", [HP, M * Adev], bf16, isOutput=False)
    out_d = nc.declare_dram_parameter(
        "out", [n_pair, Adev, 2 * SUPER], bf16, isOutput=True
    )

    xa = nc.alloc_sbuf_tensor("xa_sb", [64 + HP, n_grp * GCOL], bf16)
    weff = nc.alloc_sbuf_tensor("weff_sb", [64 + HP, M * Adev], bf16)
    otb = nc.alloc_sbuf_tensor("ot_sb", [Adev, n_super * SUPER], bf16)
    ots = [otb[:, s * SUPER : (s + 1) * SUPER] for s in range(n_super)]
    psos = [nc.alloc_psum_tensor(f"pso{k}", [Adev, SUPER], f32) for k in range(NP)]

    NSX = 4  # rotating input-load sems
    NSQ = 4  # rotating store sems
    sem_w = [nc.alloc_semaphore(f"sem_w{k}") for k in range(2)]
    sem_x = [nc.alloc_semaphore(f"sem_x{k}") for k in range(NSX)]
    sem_mm = nc.alloc_semaphore("sem_mm")
    sem_cv = nc.alloc_semaphore("sem_cv")
    sem_ca = nc.alloc_semaphore("sem_ca")
    sem_oe = [nc.alloc_semaphore(f"sem_oe{k}") for k in range(NSQ)]

    with nc.Block() as block:

        @block.gpsimd
        def _(g):
            for gi in range(1, n_grp):
                cols = slice(gi * GCOL, (gi + 1) * GCOL)
                sx = sem_x[gi % NSX]
                if gi >= NSX:
                    g.wait_ge(sx, 32 * (gi // NSX))
                g.dma_start(xa[0:HP, cols], xat_d[gi, 0]).then_inc(sx, 16)
                g.dma_start(xa[64 : 64 + HP, cols], xat_d[gi, 1]).then_inc(sx, 16)

        # cast-engine assignment: DVE takes even supers, ACT takes odd supers
        # plus the final odd-count super (ACT is ~9% faster per op)
        dve_set = [sc for sc in range(0, n_super - (n_super % 2), 2)]
        act_set = [sc for sc in range(1, n_super, 2)] + (
            [n_super - 1] if n_super % 2 == 1 else []
        )
        dve_rank = {sc: i + 1 for i, sc in enumerate(dve_set)}
        act_rank = {sc: i + 1 for i, sc in enumerate(act_set)}

        def wait_cast_done(eng, k):
            if k in dve_rank:
                eng.wait_ge(sem_cv, dve_rank[k])
            else:
                eng.wait_ge(sem_ca, act_rank[k])

        @block.tensor
        def _(t):
            t.wait_ge(sem_w[0], 16)
            t.wait_ge(sem_w[1], 16)
            for sc in range(n_super):
                gi, j = divmod(sc, GRP)
                if sc % GRP == 0:
                    t.wait_ge(sem_x[gi % NSX], 32 * (gi // NSX + 1))
                if sc >= NP:
                    wait_cast_done(t, sc - NP)
                pso = psos[sc % NP]
                mms = []
                for h in range(2):
                    base = 0 if h == 0 else 64
                    for (m, a, b) in runs[sc * 2 + h]:
                        c0 = a - sc * SUPER
                        c1 = b - sc * SUPER
                        xcol = gi * GCOL + j * HALF
                        mms.append(
                            t.matmul(
                                pso[:, c0:c1],
                                weff[base : base + HP, m * Adev : (m + 1) * Adev],
                                xa[
                                    base : base + HP,
                                    xcol + c0 - h * HALF : xcol + c1 - h * HALF,
                                ],
                                start=True,
                                stop=True,
                            )
                        )
                mms[-1].then_inc(sem_mm, 1)

        @block.vector
        def _(v):
            for sc in dve_set:
                v.wait_ge(sem_mm, sc + 1)
                v.tensor_copy(ots[sc][:], psos[sc % NP][:]).then_inc(sem_cv, 1)

        @block.scalar
        def _(s):
            # group-0 low half on the scalar HWDGE ring: runs in parallel with
            # the sync ring's loads while the gpsimd SWDGE is still booting
            s.dma_start(xa[0:HP, 0:GCOL], xat_d[0, 0]).then_inc(sem_x[0], 16)
            for sc in act_set:
                s.wait_ge(sem_mm, sc + 1)
                s.copy(ots[sc][:], psos[sc % NP][:]).then_inc(sem_ca, 1)

        @block.sync
        def _(sy):
            sy.dma_start(weff[0:HP, :], weff_d[:]).then_inc(sem_w[0], 16)
            sy.dma_start(weff[64 : 64 + HP, :], weff_d[:]).then_inc(sem_w[1], 16)
            sy.dma_start(xa[64 : 64 + HP, 0:GCOL], xat_d[0, 1]).then_inc(sem_x[0], 16)
            for p in range(n_pair):
                c = 2 * SUPER if 2 * p + 1 < n_super else SUPER
                wait_cast_done(sy, 2 * p)
                if 2 * p + 1 < n_super:
                    wait_cast_done(sy, 2 * p + 1)
                so = sem_oe[p % NSQ]
                if p >= NSQ:
                    sy.wait_ge(so, 16 * (p // NSQ))
                sy.dma_start(
                    out_d[p][:, 0:c], otb[:, 2 * p * SUPER : 2 * p * SUPER + c]
                ).then_inc(so, 16)
            for k in range(NSQ):
                cnt = (n_pair - 1 - k) // NSQ + 1 if k < n_pair else 0
                if cnt:
                    sy.wait_ge(sem_oe[k], 16 * cnt)

    nc.compile()
    return nc


def kernel(states, epoch_idx, W1, b1, Wout, bout, mask):
    global LAST_RESULT
    from concourse.bass_utils import run_bass_kernel_spmd

    states = np.asarray(states, dtype=np.float32)
    epoch_idx = np.asarray(epoch_idx, dtype=np.int32)
    W1 = np.asarray(W1, dtype=np.float32)
    b1 = np.asarray(b1, dtype=np.float32)
    Wout = np.asarray(Wout, dtype=np.float32)
    bout = np.asarray(bout, dtype=np.float32)
    mask = np.asarray(mask, dtype=np.int32)

    keep = mask.reshape(A) != 0
    kept_cols = np.nonzero(keep)[0]
    Ak = int(len(kept_cols))
    if Ak == 0:
        return np.full((B, J, J), NEG, np.float32)
    Adev = min(Ak, 128)
    dev_cols = kept_cols[:Adev]
    rem_cols = kept_cols[Adev:]

    # --- shared trunk on host (tiny: ~0.6 GFLOP BLAS) ---
    x = np.maximum(states @ W1.T + b1[None, :], 0.0)  # [B, H] f32

    # --- route rows: per expert, deal round-robin across cores ---
    core_idx = [[None] * M for _ in range(NCORES)]
    for m in range(M):
        idx_m = np.nonzero(epoch_idx == m)[0]
        for i in range(NCORES):
            core_idx[i][m] = idx_m[i::NCORES]
    cnt = [[len(core_idx[i][m]) for m in range(M)] for i in range(NCORES)]
    # shared per-expert row capacity across cores (row-granular)
    caps = [max(cnt[i][m] for i in range(NCORES)) for m in range(M)]
    need = sum(caps)
    R = SUPER * ((max(need, B // NCORES) + SUPER - 1) // SUPER)
    caps[-1] += R - need  # dump slack into the last expert
    caps = tuple(caps)
    offs = np.concatenate([[0], np.cumsum(caps)])

    # --- effective expert weights (device columns only; bout in ones row) ---
    weff = np.zeros((HP, M * Adev), np.float32)
    for m in range(M):
        weff[:H, m * Adev : (m + 1) * Adev] = Wout[m][dev_cols].T
        weff[H, m * Adev : (m + 1) * Adev] = bout[m][dev_cols]
    weff_bf = weff.astype(BF16)

    # --- pack per-core transposed activations (bf16, group-major) ---
    n_super = R // SUPER
    GRP = 3 if n_super % 3 == 0 else (2 if n_super % 2 == 0 else 1)
    in_maps = []
    for i in range(NCORES):
        packed = np.zeros((R, HP), np.float32)
        packed[:, H] = 1.0  # ones row for bias folding
        for m in range(M):
            r0 = int(offs[m])
            packed[r0 : r0 + cnt[i][m], :H] = x[core_idx[i][m]]
        xat = np.ascontiguousarray(
            packed.astype(BF16)
            .reshape(n_super // GRP, GRP, 2, HALF, HP)
            .transpose(0, 2, 4, 1, 3)
            .reshape(n_super // GRP, 2, HP, GRP * HALF)
        )
        in_maps.append({"xat": xat, "weff": weff_bf})

    key = (R, caps, Adev)
    nc = _BUILD_CACHE.get(key)
    if nc is None:
        nc = _build(R, caps, Adev)
        _BUILD_CACHE[key] = nc

    # retry: rare transient NRT_EXEC_UNIT_UNRECOVERABLE on fresh NEFFs
    last_err = None
    for _attempt in range(3):
        try:
            res = run_bass_kernel_spmd(nc, in_maps, core_ids=list(range(NCORES)))
            break
        except Exception as e:  # noqa: BLE001
            last_err = e
    else:
        raise last_err
    LAST_RESULT = res

    # --- unpack: [n_pair, Adev, 2048] -> rows [R, Adev] ---
    out_kept = np.empty((B, Adev), np.float32)
    for i in range(NCORES):
        oc = np.asarray(res.results[i]["out"])
        rows = oc.transpose(0, 2, 1).reshape(-1, Adev)[:R].astype(np.float32)
        for m in range(M):
            r0 = int(offs[m])
            out_kept[core_idx[i][m]] = rows[r0 : r0 + cnt[i][m]]

    out_full = np.full((B, A), NEG, np.float32)
    out_full[:, dev_cols] = out_kept

    # --- host remainder: kept columns beyond the device's 128 ---
    if len(rem_cols):
        for m in range(M):
            rows_m = np.nonzero(epoch_idx == m)[0]
            out_full[rows_m[:, None], rem_cols[None, :]] = (
                x[rows_m] @ Wout[m][rem_cols].T + bout[m][rem_cols][None, :]
            )

    return out_full.reshape(B, J, J)


# revision 37
# speedup vs baseline: 1.0856x; 1.0856x over previous
"""MoE-routing actor kernel for 8 Trainium2 NeuronCores.

Strategy (pure data parallel, expert-sorted, bf16 compute):
  - Host: for each expert m, deal its rows round-robin to the 8 cores so all
    cores get near-identical per-expert counts and can share ONE SPMD graph.
    Per-expert row capacities are the max count over cores (row-granular,
    <1% padding); rows are packed sorted-by-expert.
  - The tiny shared trunk (fc1: 262144x32 @ 32x34 + relu, ~0.6 GFLOP) runs on
    host BLAS; the device gets pre-packed transposed activations with an
    all-ones row 34 that folds the expert bias bout into the expert matmul.
  - Mask applied host-side: the device computes only the first A_DEV (<=128)
    kept output columns; masked columns are exact -1e9 filled host-side and
    kept columns beyond A_DEV (typically ~9 of 137) are computed on host.
  - Device (raw bacc, manual semaphores, no Tile framework): per 1024-row
    super-chunk, transposed expert matmuls (stationary weff_e [35, A_DEV],
    moving activation run of <=512 rows, expert-boundary runs split).
    Consecutive 512-row halves alternate PE partition base 0/64 so each
    LDWEIGHTS targets row-strips disjoint from the in-flight matmul AND the
    two halves' matmuls execute concurrently on disjoint sub-arrays
    (~0.5 PE cycles/row at the fixed 1.2 GHz clock).  PSUM->bf16 casts
    alternate VectorE/ScalarE per super; stores go out in 512KB pairs on the
    sync HWDGE ring; input loads stream on the gpsimd queue (group 0 on the
    scalar ring to dodge SWDGE boot latency).
"""

import os
import sys

sys.path.insert(0, "/opt/trn_rl_repo")

import numpy as np
import ml_dtypes

BF16 = ml_dtypes.bfloat16

B = 262144
NCORES = 8
J = 16
M = 12
H = 34
HP = H + 1  # fc1 output + ones row for bias folding
S_DIM = 32  # state dim
A = J * J  # 256 action logits
NEG = np.float32(-1.0e9)
SUPER = 1024  # rows per compute chunk
HALF = 512  # PSUM-bank / matmul free-dim granule
NP = 4  # psum ring depth (supers)

_BUILD_CACHE: dict = {}
LAST_RESULT = None  # BassKernelResults of the most recent run (for profiling)


def _make_runs(caps, R):
    """Per 512-row half-chunk, the (expert, row0, row1) runs covering it."""
    offs = np.concatenate([[0], np.cumsum(caps)])
    assert offs[-1] == R
    runs = [[] for _ in range(R // HALF)]
    for m in range(len(caps)):
        lo, hi = int(offs[m]), int(offs[m + 1])
        if lo >= hi:
            continue
        for g in range(lo // HALF, (hi - 1) // HALF + 1):
            a = max(lo, g * HALF)
            b = min(hi, (g + 1) * HALF)
            if a < b:
                runs[g].append((m, a, b))
    return runs


def _build(R: int, caps: tuple, Adev: int):
    """Raw-bacc device graph: manual semaphores, static SBUF allocation.

    R is the number of LIVE rows (multiple of 512); the input layout is
    padded to whole 1024-row supers, but the final half-super (if R is an
    odd multiple of 512) gets no matmuls, a half-width cast and store.
    """
    from concourse import bacc, mybir

    n_half = R // HALF
    n_super = (n_half + 1) // 2
    runs = _make_runs(list(caps), R)
    runs += [[]] * (2 * n_super - len(runs))  # dead trailing half
    f32 = mybir.dt.float32
    bf16 = mybir.dt.bfloat16
    nc = bacc.Bacc("TRN2", target_bir_lowering=False, debug=False)

    GRP = 3 if n_super % 3 == 0 else (2 if n_super % 2 == 0 else 1)
    n_grp = n_super // GRP
    GCOL = GRP * HALF
    n_pair = (n_super + 1) // 2

    def super_cols(sc):  # live columns of super sc
        return HALF if 2 * sc + 1 >= n_half else SUPER

    xat_d = nc.declare_dram_parameter("xat", [n_grp, 2, HP, GCOL], bf16, isOutput=False)
    weff_d = nc.declare_dram_parameter("weff", [HP, M * Adev], bf16, isOutput=False)
    out_d = nc.declare_dram_parameter(
        "out", [n_pair, Adev, 2 * SUPER], bf16, isOutput=True
    )

    xa = nc.alloc_sbuf_tensor("xa_sb", [64 + HP, n_grp * GCOL], bf16)
    weff = nc.alloc_sbuf_tensor("weff_sb", [64 + HP, M * Adev], bf16)
    otb = nc.alloc_sbuf_tensor("ot_sb", [Adev, n_super * SUPER], bf16)
    ots = [otb[:, s * SUPER : (s + 1) * SUPER] for s in range(n_super)]
    psos = [nc.alloc_psum_tensor(f"pso{k}", [Adev, SUPER], f32) for k in range(NP)]

    NSX = 4  # rotating input-load sems
    NSQ = 4  # rotating store sems
    sem_w = [nc.alloc_semaphore(f"sem_w{k}") for k in range(2)]
    sem_x = [nc.alloc_semaphore(f"sem_x{k}") for k in range(NSX)]
    sem_mm = nc.alloc_semaphore("sem_mm")
    sem_cv = nc.alloc_semaphore("sem_cv")
    sem_ca = nc.alloc_semaphore("sem_ca")
    sem_oe = [nc.alloc_semaphore(f"sem_oe{k}") for k in range(NSQ)]

    with nc.Block() as block:

        @block.gpsimd
        def _(g):
            for gi in range(1, n_grp):
                cols = slice(gi * GCOL, (gi + 1) * GCOL)
                sx = sem_x[gi % NSX]
                if gi >= NSX:
                    g.wait_ge(sx, 32 * (gi // NSX))
                g.dma_start(xa[0:HP, cols], xat_d[gi, 0]).then_inc(sx, 16)
                g.dma_start(xa[64 : 64 + HP, cols], xat_d[gi, 1]).then_inc(sx, 16)

        # cast-engine assignment: DVE takes even supers, ACT takes odd supers
        # plus the final odd-count super (ACT is ~9% faster per op)
        dve_set = [sc for sc in range(0, n_super - (n_super % 2), 2)]
        act_set = [sc for sc in range(1, n_super, 2)] + (
            [n_super - 1] if n_super % 2 == 1 else []
        )
        dve_rank = {sc: i + 1 for i, sc in enumerate(dve_set)}
        act_rank = {sc: i + 1 for i, sc in enumerate(act_set)}

        def wait_cast_done(eng, k):
            if k in dve_rank:
                eng.wait_ge(sem_cv, dve_rank[k])
            else:
                eng.wait_ge(sem_ca, act_rank[k])

        @block.tensor
        def _(t):
            t.wait_ge(sem_w[0], 16)
            t.wait_ge(sem_w[1], 16)
            for sc in range(n_super):
                gi, j = divmod(sc, GRP)
                if sc % GRP == 0:
                    t.wait_ge(sem_x[gi % NSX], 32 * (gi // NSX + 1))
                if sc >= NP:
                    wait_cast_done(t, sc - NP)
                pso = psos[sc % NP]
                mms = []
                for h in range(2):
                    base = 0 if h == 0 else 64
                    for (m, a, b) in runs[sc * 2 + h]:
                        c0 = a - sc * SUPER
                        c1 = b - sc * SUPER
                        xcol = gi * GCOL + j * HALF
                        mms.append(
                            t.matmul(
                                pso[:, c0:c1],
                                weff[base : base + HP, m * Adev : (m + 1) * Adev],
                                xa[
                                    base : base + HP,
                                    xcol + c0 - h * HALF : xcol + c1 - h * HALF,
                                ],
                                start=True,
                                stop=True,
                            )
                        )
                mms[-1].then_inc(sem_mm, 1)

        @block.vector
        def _(v):
            for sc in dve_set:
                c = super_cols(sc)
                v.wait_ge(sem_mm, sc + 1)
                v.tensor_copy(
                    ots[sc][:, 0:c], psos[sc % NP][:, 0:c]
                ).then_inc(sem_cv, 1)

        @block.scalar
        def _(s):
            # group-0 low half on the scalar HWDGE ring: runs in parallel with
            # the sync ring's loads while the gpsimd SWDGE is still booting
            s.dma_start(xa[0:HP, 0:GCOL], xat_d[0, 0]).then_inc(sem_x[0], 16)
            for sc in act_set:
                c = super_cols(sc)
                s.wait_ge(sem_mm, sc + 1)
                s.copy(ots[sc][:, 0:c], psos[sc % NP][:, 0:c]).then_inc(sem_ca, 1)

        @block.sync
        def _(sy):
            sy.dma_start(weff[0:HP, :], weff_d[:]).then_inc(sem_w[0], 16)
            sy.dma_start(weff[64 : 64 + HP, :], weff_d[:]).then_inc(sem_w[1], 16)
            sy.dma_start(xa[64 : 64 + HP, 0:GCOL], xat_d[0, 1]).then_inc(sem_x[0], 16)
            for p in range(n_pair):
                c = super_cols(2 * p)
                if 2 * p + 1 < n_super:
                    c += super_cols(2 * p + 1)
                wait_cast_done(sy, 2 * p)
                if 2 * p + 1 < n_super:
                    wait_cast_done(sy, 2 * p + 1)
                so = sem_oe[p % NSQ]
                if p >= NSQ:
                    sy.wait_ge(so, 16 * (p // NSQ))
                sy.dma_start(
                    out_d[p][:, 0:c], otb[:, 2 * p * SUPER : 2 * p * SUPER + c]
                ).then_inc(so, 16)
            for k in range(NSQ):
                cnt = (n_pair - 1 - k) // NSQ + 1 if k < n_pair else 0
                if cnt:
                    sy.wait_ge(sem_oe[k], 16 * cnt)

    nc.compile()
    return nc


def kernel(states, epoch_idx, W1, b1, Wout, bout, mask):
    global LAST_RESULT
    from concourse.bass_utils import run_bass_kernel_spmd

    states = np.asarray(states, dtype=np.float32)
    epoch_idx = np.asarray(epoch_idx, dtype=np.int32)
    W1 = np.asarray(W1, dtype=np.float32)
    b1 = np.asarray(b1, dtype=np.float32)
    Wout = np.asarray(Wout, dtype=np.float32)
    bout = np.asarray(bout, dtype=np.float32)
    mask = np.asarray(mask, dtype=np.int32)

    keep = mask.reshape(A) != 0
    kept_cols = np.nonzero(keep)[0]
    Ak = int(len(kept_cols))
    if Ak == 0:
        return np.full((B, J, J), NEG, np.float32)
    Adev = min(Ak, 128)
    dev_cols = kept_cols[:Adev]
    rem_cols = kept_cols[Adev:]

    # --- shared trunk on host (tiny: ~0.6 GFLOP BLAS) ---
    x = np.maximum(states @ W1.T + b1[None, :], 0.0)  # [B, H] f32

    # --- route rows: per expert, deal round-robin across cores ---
    core_idx = [[None] * M for _ in range(NCORES)]
    for m in range(M):
        idx_m = np.nonzero(epoch_idx == m)[0]
        for i in range(NCORES):
            core_idx[i][m] = idx_m[i::NCORES]
    cnt = [[len(core_idx[i][m]) for m in range(M)] for i in range(NCORES)]
    # shared per-expert row capacity across cores (row-granular)
    caps = [max(cnt[i][m] for i in range(NCORES)) for m in range(M)]
    need = sum(caps)
    R = HALF * ((max(need, B // NCORES) + HALF - 1) // HALF)
    caps[-1] += R - need  # dump slack into the last expert
    caps = tuple(caps)
    offs = np.concatenate([[0], np.cumsum(caps)])

    # --- effective expert weights (device columns only; bout in ones row) ---
    weff = np.zeros((HP, M * Adev), np.float32)
    for m in range(M):
        weff[:H, m * Adev : (m + 1) * Adev] = Wout[m][dev_cols].T
        weff[H, m * Adev : (m + 1) * Adev] = bout[m][dev_cols]
    weff_bf = weff.astype(BF16)

    # --- pack per-core transposed activations (bf16, group-major) ---
    n_super = (R // HALF + 1) // 2
    R_grid = n_super * SUPER
    GRP = 3 if n_super % 3 == 0 else (2 if n_super % 2 == 0 else 1)
    in_maps = []
    for i in range(NCORES):
        packed = np.zeros((R_grid, HP), np.float32)
        packed[:, H] = 1.0  # ones row for bias folding
        for m in range(M):
            r0 = int(offs[m])
            packed[r0 : r0 + cnt[i][m], :H] = x[core_idx[i][m]]
        xat = np.ascontiguousarray(
            packed.astype(BF16)
            .reshape(n_super // GRP, GRP, 2, HALF, HP)
            .transpose(0, 2, 4, 1, 3)
            .reshape(n_super // GRP, 2, HP, GRP * HALF)
        )
        in_maps.append({"xat": xat, "weff": weff_bf})

    key = (R, caps, Adev)
    nc = _BUILD_CACHE.get(key)
    if nc is None:
        nc = _build(R, caps, Adev)
        _BUILD_CACHE[key] = nc

    # retry: rare transient NRT_EXEC_UNIT_UNRECOVERABLE on fresh NEFFs
    last_err = None
    for _attempt in range(3):
        try:
            res = run_bass_kernel_spmd(nc, in_maps, core_ids=list(range(NCORES)))
            break
        except Exception as e:  # noqa: BLE001
            last_err = e
    else:
        raise last_err
    LAST_RESULT = res

    # --- unpack: [n_pair, Adev, 2048] -> rows [R, Adev] ---
    out_kept = np.empty((B, Adev), np.float32)
    for i in range(NCORES):
        oc = np.asarray(res.results[i]["out"])
        rows = oc.transpose(0, 2, 1).reshape(-1, Adev)[:R].astype(np.float32)
        for m in range(M):
            r0 = int(offs[m])
            out_kept[core_idx[i][m]] = rows[r0 : r0 + cnt[i][m]]

    out_full = np.full((B, A), NEG, np.float32)
    out_full[:, dev_cols] = out_kept

    # --- host remainder: kept columns beyond the device's 128 ---
    if len(rem_cols):
        for m in range(M):
            rows_m = np.nonzero(epoch_idx == m)[0]
            out_full[rows_m[:, None], rem_cols[None, :]] = (
                x[rows_m] @ Wout[m][rem_cols].T + bout[m][rem_cols][None, :]
            )

    return out_full.reshape(B, J, J)
